# revision 43
# baseline (speedup 1.0000x reference)
"""Trainium2 Bass kernel for nn_BasicModel_47871705481510.

Math: per qubit i, with v_i = w_i + (x[0] if i==0 else x[1] if i==1 else 0):
  state_i = [cos(v_i/2), -i*sin(v_i/2)]^T   (Rx(w) @ Rx(theta1) |0> collapses
                                             to a single rotation by v_i)
  O_i     = cos(v_i)                         (real)

Device computes three f32 planes per qubit. The ACT Sin table is accurate
only for |arg| <= pi (max |v| here is ~5.42), so:
  sneg = sin(-v/2)                 |v/2| <= 2.71 < pi         (ACT)
  c    = sin(pi/2 - |v|/2)         arg in [-1.14, pi/2]       (DVE sign-clear
                                                               + ACT)
  cosv = 1 - 2*sneg^2                                          (DVE)
Host assembles the complex64 outputs (structural zeros / interleave only).

Sharding: data-parallel over qubits. 2,000,000 qubits padded to 8 * 128 * 2048
= 2,097,152; each of the 8 cores handles one contiguous [128, 2048] f32 shard.
x is passed as a [1,2] tensor: the real x to core 0 (whose shard holds qubits
0,1 at partition 0, cols 0,1), zeros to the other cores, so the SPMD program
is uniform. A [128,1] pi/2 bias column is shipped as a tiny constant input.
"""

import numpy as np

N = 2_000_000
N_CORES = 8
P = 128
F = 2048
PER_CORE = P * F  # 262144
N_PAD = PER_CORE * N_CORES

# compute chunks (columns); processed in ORDER so that chunk 0 (the only one
# needing the x-add, whose tiny DMA chain wakes slowly) runs last
CHS = [128, 512, 704, 704]
ORDER = [1, 2, 3, 0]

_cache = {}


def _build_nc_raw():
    """Raw Bacc kernel (no TileContext): hand-placed semaphores, distinct
    SBUF buffers (no reuse -> only RAW deps need sems), no Tile tail
    machinery. Streams:
      Sync:   pi/2-bias + x DMAs first (tiny, warms this queue), then the
              first two processed in-chunks, then o-plane outs
      Scalar: warm-up Sin (hoists ACT_TABLE_LOAD), remaining in-chunks,
              all ACT ops, c-plane outs
      Pool:   s-plane outs (SWDGE queue)
      Vector: per chunk: |v| sign-clear, sneg^2, cosv affine; the x add
    Outputs stream out per chunk as soon as each plane piece is computed.
    """
    import concourse.bacc as bacc
    import concourse.mybir as mybir

    nc = bacc.Bacc(
        "TRN2", target_bir_lowering=False, debug=False, num_devices=N_CORES
    )
    w_in = nc.declare_dram_parameter("w", [P, F], mybir.dt.float32, isOutput=False)
    x_in = nc.declare_dram_parameter("x", [1, 2], mybir.dt.float32, isOutput=False)
    c_out = nc.declare_dram_parameter("c", [P, F], mybir.dt.float32, isOutput=True)
    s_out = nc.declare_dram_parameter("s", [P, F], mybir.dt.float32, isOutput=True)
    o_out = nc.declare_dram_parameter("o", [P, F], mybir.dt.float32, isOutput=True)

    SIN = mybir.ActivationFunctionType.Sin
    MULT = mybir.AluOpType.mult
    ADD = mybir.AluOpType.add
    AND = mybir.AluOpType.bitwise_and
    f32 = mybir.dt.float32
    i32 = mybir.dt.int32

    NCH = len(CHS)
    offs = [sum(CHS[:j]) for j in range(NCH)]
    assert sum(CHS) == F
    assert sorted(ORDER) == list(range(NCH)) and ORDER[-1] == 0
    # processing position of chunk j
    pos = {j: p for p, j in enumerate(ORDER)}

    wt = [nc.alloc_sbuf_tensor(f"wt{j}", [P, CHS[j]], f32).ap() for j in range(NCH)]
    at = [nc.alloc_sbuf_tensor(f"at{j}", [P, CHS[j]], f32).ap() for j in range(NCH)]
    s2 = [nc.alloc_sbuf_tensor(f"s2{j}", [P, CHS[j]], f32).ap() for j in range(NCH)]
    st = [nc.alloc_sbuf_tensor(f"st{j}", [P, CHS[j]], f32).ap() for j in range(NCH)]
    ct = [nc.alloc_sbuf_tensor(f"ct{j}", [P, CHS[j]], f32).ap() for j in range(NCH)]
    ot = [nc.alloc_sbuf_tensor(f"ot{j}", [P, CHS[j]], f32).ap() for j in range(NCH)]
    xt = nc.alloc_sbuf_tensor("xt", [1, 2], f32).ap()
    bt = nc.alloc_sbuf_tensor("bt", [P, 1], f32).ap()
    warm = nc.alloc_sbuf_tensor("warm", [1, 1], f32).ap()
    zconst = nc.const_aps.tensor(0.0, (1, 1), f32)

    in_sl = [slice(offs[j], offs[j] + CHS[j]) for j in range(NCH)]

    # semaphore counts by processing position p (chunk j = ORDER[p]):
    #   act: st -> 2p+1, c -> 2p+2
    # DVE stream: ANDs for chunks ORDER[:-1] hoisted to the front (they only
    # need input data); the x add + AND_0 slot in right after the first
    # [s2, o] pair so chunk 0's ACT work can append immediately after the
    # other chunks'; remaining [s2, o] pairs follow.
    DVE_SEQ = (
        [("and", j) for j in ORDER[:-1]]
        + [("s2o", ORDER[0]), ("addand0", 0)]
        + [("s2o", j) for j in ORDER[1:-1]]
        + [("s2o", 0)]
    )
    DAND = {}
    DS2 = {}
    DOT = {}
    _d = 0
    for kind, j in DVE_SEQ:
        if kind == "and":
            _d += 1
            DAND[j] = _d
        elif kind == "addand0":
            _d += 1
            DAND[0] = _d
        else:
            _d += 1
            DS2[j] = _d
            _d += 1
            DOT[j] = _d

    from contextlib import ExitStack

    with ExitStack() as stack:
        in_sem = [stack.enter_context(nc.semaphore(f"in{j}")) for j in range(NCH)]
        xt_sem = stack.enter_context(nc.semaphore("xt_sem"))
        bt_sem = stack.enter_context(nc.semaphore("bt_sem"))
        act_sem = stack.enter_context(nc.semaphore("act_sem"))
        add_sem = stack.enter_context(nc.semaphore("add_sem"))
        dve_sem = stack.enter_context(nc.semaphore("dve_sem"))
        out_sc = stack.enter_context(nc.semaphore("out_sc"))
        out_sy = stack.enter_context(nc.semaphore("out_sy"))
        out_po = stack.enter_context(nc.semaphore("out_po"))
        block = stack.enter_context(nc.Block())

        @block.sync
        def _(sync):
            # first processed in-chunk FIRST: it gates the whole pipeline;
            # the tiny x transfer rides behind it
            sync.dma_start(
                wt[ORDER[0]], w_in[:, in_sl[ORDER[0]]]
            ).then_inc(in_sem[ORDER[0]], 16)
            sync.dma_start(xt, x_in[:]).then_inc(xt_sem, 16)
            for p, j in enumerate(ORDER):
                sync.wait_ge(dve_sem, DOT[j])
                sync.dma_start(o_out[:, in_sl[j]], ot[j]).then_inc(out_sy, 16)
            sync.wait_ge(out_sy, 16 * NCH)

        @block.scalar
        def _(scalar):
            scalar.activation(warm, zconst, SIN)  # pulls ACT_TABLE_LOAD early
            for j in ORDER[1:]:
                scalar.dma_start(wt[j], w_in[:, in_sl[j]]).then_inc(in_sem[j], 16)
            for p, j in enumerate(ORDER):
                scalar.wait_ge(in_sem[j], 16)
                if j == 0:
                    scalar.wait_ge(add_sem, 1)
                scalar.activation(st[j], wt[j], SIN, scale=-0.5).then_inc(act_sem, 1)
                if p == 0:
                    scalar.wait_ge(bt_sem, 1)
                scalar.wait_ge(dve_sem, DAND[j])  # |v| ready
                scalar.activation(ct[j], at[j], SIN, bias=bt, scale=-0.5).then_inc(
                    act_sem, 1
                )
                if p >= 1:
                    # previous chunk's c piece out: the DIRECT2D (~0.65us)
                    # hides under this chunk's ACT execution
                    jprev = ORDER[p - 1]
                    scalar.wait_ge(act_sem, 2 * p)
                    scalar.dma_start(
                        c_out[:, in_sl[jprev]], ct[jprev]
                    ).then_inc(out_sc, 16)
            scalar.wait_ge(act_sem, 2 * NCH)
            scalar.dma_start(
                c_out[:, in_sl[ORDER[-1]]], ct[ORDER[-1]]
            ).then_inc(out_sc, 16)
            scalar.wait_ge(out_sc, 16 * NCH)

        @block.gpsimd
        def _(gpsimd):
            # pi/2 bias column produced locally -- no DMA, no latency chain
            gpsimd.memset(bt, float(np.pi / 2)).then_inc(bt_sem, 1)
            for p, j in enumerate(ORDER):
                gpsimd.wait_ge(act_sem, 2 * p + 1)
                gpsimd.dma_start(s_out[:, in_sl[j]], st[j]).then_inc(out_po, 16)
            gpsimd.wait_ge(out_po, 16 * NCH)

        @block.vector
        def _(vector):
            for kind, j in DVE_SEQ:
                if kind == "and":
                    vector.wait_ge(in_sem[j], 16)
                    vector.tensor_scalar(
                        at[j].bitcast(i32), wt[j].bitcast(i32), 0x7FFFFFFF, None, AND
                    ).then_inc(dve_sem, 1)
                elif kind == "addand0":
                    vector.wait_ge(xt_sem, 16)
                    vector.wait_ge(in_sem[0], 16)
                    vector.tensor_add(
                        wt[0][0:1, 0:2], wt[0][0:1, 0:2], xt[0:1, 0:2]
                    ).then_inc(add_sem, 1)
                    vector.wait_ge(add_sem, 1)  # same-engine RAW via DVE pipe
                    vector.tensor_scalar(
                        at[0].bitcast(i32), wt[0].bitcast(i32), 0x7FFFFFFF, None, AND
                    ).then_inc(dve_sem, 1)
                else:
                    vector.wait_ge(act_sem, 2 * pos[j] + 1)  # st_j ready
                    vector.tensor_mul(s2[j], st[j], st[j]).then_inc(dve_sem, 1)
                    vector.wait_ge(dve_sem, DS2[j])  # same-engine RAW (s2)
                    vector.tensor_scalar(
                        ot[j], s2[j], -2.0, 1.0, MULT, ADD
                    ).then_inc(dve_sem, 1)

    nc.finalize()
    return nc


def _get_nc():
    if "nc" not in _cache:
        _cache["nc"] = _build_nc_raw()
    return _cache["nc"]


def _run(x, w, **spmd_kwargs):
    """Shard, run on 8 cores, return (c, sneg, cosv) full f32 vectors plus
    the raw BassKernelResults (for profiling from test harnesses)."""
    from concourse.bass_utils import run_bass_kernel_spmd

    x = np.ascontiguousarray(np.asarray(x, dtype=np.float32)).reshape(1, 2)
    w = np.asarray(w, dtype=np.float32).reshape(-1)
    assert w.shape[0] == N
    w_pad = np.zeros(N_PAD, dtype=np.float32)
    w_pad[:N] = w
    shards = w_pad.reshape(N_CORES, P, F)
    zero_x = np.zeros((1, 2), dtype=np.float32)
    in_maps = [
        {"w": shards[i], "x": (x if i == 0 else zero_x)} for i in range(N_CORES)
    ]
    res = run_bass_kernel_spmd(_get_nc(), in_maps, list(range(N_CORES)), **spmd_kwargs)
    c = np.concatenate([r["c"].reshape(-1) for r in res.results])[:N]
    sneg = np.concatenate([r["s"].reshape(-1) for r in res.results])[:N]
    cosv = np.concatenate([r["o"].reshape(-1) for r in res.results])[:N]
    return c, sneg, cosv, res


def kernel(x, w):
    c, sneg, cosv, _ = _run(x, w)
    state = np.zeros((N, 4), dtype=np.float32)
    state[:, 0] = c
    state[:, 3] = sneg
    state = state.view(np.complex64).reshape(N, 2, 1)
    O = np.zeros((N, 2), dtype=np.float32)
    O[:, 0] = cosv
    O = O.view(np.complex64).reshape(N, 1, 1)
    return state, O


# revision 45
# speedup vs baseline: 1.0047x; 1.0047x over previous
"""Trainium2 Bass kernel for nn_BasicModel_47871705481510.

Math: per qubit i, with v_i = w_i + (x[0] if i==0 else x[1] if i==1 else 0):
  state_i = [cos(v_i/2), -i*sin(v_i/2)]^T   (Rx(w) @ Rx(theta1) |0> collapses
                                             to a single rotation by v_i)
  O_i     = cos(v_i)                         (real)

Device computes three f32 planes per qubit. The ACT Sin table is accurate
only for |arg| <= pi (max |v| here is ~5.42), so:
  sneg = sin(-v/2)                 |v/2| <= 2.71 < pi         (ACT)
  c    = sin(pi/2 - |v|/2)         arg in [-1.14, pi/2]       (DVE sign-clear
                                                               + ACT)
  cosv = 1 - 2*sneg^2                                          (DVE)
Host assembles the complex64 outputs (structural zeros / interleave only).

Sharding: data-parallel over qubits. 2,000,000 qubits padded to 8 * 128 * 2048
= 2,097,152; each of the 8 cores handles one contiguous [128, 2048] f32 shard.
x is passed as a [1,2] tensor: the real x to core 0 (whose shard holds qubits
0,1 at partition 0, cols 0,1), zeros to the other cores, so the SPMD program
is uniform. A [128,1] pi/2 bias column is shipped as a tiny constant input.
"""

import numpy as np

N = 2_000_000
N_CORES = 8
P = 128
F = 2048
PER_CORE = P * F  # 262144
N_PAD = PER_CORE * N_CORES

# compute chunks (columns); processed in ORDER so that chunk 0 (the only one
# needing the x-add, whose tiny DMA chain wakes slowly) runs last
CHS = [128, 512, 704, 704]
ORDER = [1, 2, 3, 0]

_cache = {}


def _build_nc_raw():
    """Raw Bacc kernel (no TileContext): hand-placed semaphores, distinct
    SBUF buffers (no reuse -> only RAW deps need sems), no Tile tail
    machinery. Streams:
      Sync:   pi/2-bias + x DMAs first (tiny, warms this queue), then the
              first two processed in-chunks, then o-plane outs
      Scalar: warm-up Sin (hoists ACT_TABLE_LOAD), remaining in-chunks,
              all ACT ops, c-plane outs
      Pool:   s-plane outs (SWDGE queue)
      Vector: per chunk: |v| sign-clear, sneg^2, cosv affine; the x add
    Outputs stream out per chunk as soon as each plane piece is computed.
    """
    import concourse.bacc as bacc
    import concourse.mybir as mybir

    nc = bacc.Bacc(
        "TRN2", target_bir_lowering=False, debug=False, num_devices=N_CORES
    )
    w_in = nc.declare_dram_parameter("w", [P, F], mybir.dt.float32, isOutput=False)
    x_in = nc.declare_dram_parameter("x", [1, 2], mybir.dt.float32, isOutput=False)
    c_out = nc.declare_dram_parameter("c", [P, F], mybir.dt.float32, isOutput=True)
    s_out = nc.declare_dram_parameter("s", [P, F], mybir.dt.float32, isOutput=True)
    o_out = nc.declare_dram_parameter("o", [P, F], mybir.dt.float32, isOutput=True)

    SIN = mybir.ActivationFunctionType.Sin
    MULT = mybir.AluOpType.mult
    ADD = mybir.AluOpType.add
    AND = mybir.AluOpType.bitwise_and
    f32 = mybir.dt.float32
    i32 = mybir.dt.int32

    NCH = len(CHS)
    offs = [sum(CHS[:j]) for j in range(NCH)]
    assert sum(CHS) == F
    assert sorted(ORDER) == list(range(NCH)) and ORDER[-1] == 0
    # processing position of chunk j
    pos = {j: p for p, j in enumerate(ORDER)}

    wt = [nc.alloc_sbuf_tensor(f"wt{j}", [P, CHS[j]], f32).ap() for j in range(NCH)]
    at = [nc.alloc_sbuf_tensor(f"at{j}", [P, CHS[j]], f32).ap() for j in range(NCH)]
    s2 = [nc.alloc_sbuf_tensor(f"s2{j}", [P, CHS[j]], f32).ap() for j in range(NCH)]
    st = [nc.alloc_sbuf_tensor(f"st{j}", [P, CHS[j]], f32).ap() for j in range(NCH)]
    ct = [nc.alloc_sbuf_tensor(f"ct{j}", [P, CHS[j]], f32).ap() for j in range(NCH)]
    ot = [nc.alloc_sbuf_tensor(f"ot{j}", [P, CHS[j]], f32).ap() for j in range(NCH)]
    xt = nc.alloc_sbuf_tensor("xt", [1, 2], f32).ap()
    bt = nc.alloc_sbuf_tensor("bt", [P, 1], f32).ap()
    warm = nc.alloc_sbuf_tensor("warm", [1, 1], f32).ap()
    zconst = nc.const_aps.tensor(0.0, (1, 1), f32)

    in_sl = [slice(offs[j], offs[j] + CHS[j]) for j in range(NCH)]

    # semaphore counts by processing position p (chunk j = ORDER[p]):
    #   act: st -> 2p+1, c -> 2p+2
    # DVE stream: ANDs for chunks ORDER[:-1] hoisted to the front (they only
    # need input data); the x add + AND_0 slot in right after the first
    # [s2, o] pair so chunk 0's ACT work can append immediately after the
    # other chunks'; remaining [s2, o] pairs follow.
    DVE_SEQ = (
        [("and", j) for j in ORDER[:-1]]
        + [("s2o", ORDER[0]), ("addand0", 0)]
        + [("s2o", j) for j in ORDER[1:-1]]
        + [("s2o", 0)]
    )
    DAND = {}
    DS2 = {}
    DOT = {}
    _d = 0
    for kind, j in DVE_SEQ:
        if kind == "and":
            _d += 1
            DAND[j] = _d
        elif kind == "addand0":
            _d += 1
            DAND[0] = _d
        else:
            _d += 1
            DS2[j] = _d
            _d += 1
            DOT[j] = _d

    from contextlib import ExitStack

    with ExitStack() as stack:
        in_sem = [stack.enter_context(nc.semaphore(f"in{j}")) for j in range(NCH)]
        xt_sem = stack.enter_context(nc.semaphore("xt_sem"))
        bt_sem = stack.enter_context(nc.semaphore("bt_sem"))
        act_sem = stack.enter_context(nc.semaphore("act_sem"))
        add_sem = stack.enter_context(nc.semaphore("add_sem"))
        dve_sem = stack.enter_context(nc.semaphore("dve_sem"))
        out_sc = stack.enter_context(nc.semaphore("out_sc"))
        out_sy = stack.enter_context(nc.semaphore("out_sy"))
        out_po = stack.enter_context(nc.semaphore("out_po"))
        block = stack.enter_context(nc.Block())

        @block.sync
        def _(sync):
            # first two processed in-chunks on this (fast-waking) queue, in
            # pipeline order; the tiny x transfer rides behind them
            for j in ORDER[:2]:
                sync.dma_start(wt[j], w_in[:, in_sl[j]]).then_inc(in_sem[j], 16)
            sync.dma_start(xt, x_in[:]).then_inc(xt_sem, 16)
            for p, j in enumerate(ORDER):
                sync.wait_ge(dve_sem, DOT[j])
                sync.dma_start(o_out[:, in_sl[j]], ot[j]).then_inc(out_sy, 16)
            sync.wait_ge(out_sy, 16 * NCH)

        @block.scalar
        def _(scalar):
            scalar.activation(warm, zconst, SIN)  # pulls ACT_TABLE_LOAD early
            for j in ORDER[2:]:
                scalar.dma_start(wt[j], w_in[:, in_sl[j]]).then_inc(in_sem[j], 16)
            for p, j in enumerate(ORDER):
                scalar.wait_ge(in_sem[j], 16)
                if j == 0:
                    scalar.wait_ge(add_sem, 1)
                scalar.activation(st[j], wt[j], SIN, scale=-0.5).then_inc(act_sem, 1)
                if p == 0:
                    scalar.wait_ge(bt_sem, 1)
                scalar.wait_ge(dve_sem, DAND[j])  # |v| ready
                scalar.activation(ct[j], at[j], SIN, bias=bt, scale=-0.5).then_inc(
                    act_sem, 1
                )
                if p >= 1:
                    # previous chunk's c piece out: the DIRECT2D (~0.65us)
                    # hides under this chunk's ACT execution
                    jprev = ORDER[p - 1]
                    scalar.wait_ge(act_sem, 2 * p)
                    scalar.dma_start(
                        c_out[:, in_sl[jprev]], ct[jprev]
                    ).then_inc(out_sc, 16)
            scalar.wait_ge(act_sem, 2 * NCH)
            scalar.dma_start(
                c_out[:, in_sl[ORDER[-1]]], ct[ORDER[-1]]
            ).then_inc(out_sc, 16)
            scalar.wait_ge(out_sc, 16 * NCH)

        @block.gpsimd
        def _(gpsimd):
            # pi/2 bias column produced locally -- no DMA, no latency chain
            gpsimd.memset(bt, float(np.pi / 2)).then_inc(bt_sem, 1)
            for p, j in enumerate(ORDER):
                gpsimd.wait_ge(act_sem, 2 * p + 1)
                gpsimd.dma_start(s_out[:, in_sl[j]], st[j]).then_inc(out_po, 16)
            gpsimd.wait_ge(out_po, 16 * NCH)

        @block.vector
        def _(vector):
            for kind, j in DVE_SEQ:
                if kind == "and":
                    vector.wait_ge(in_sem[j], 16)
                    vector.tensor_scalar(
                        at[j].bitcast(i32), wt[j].bitcast(i32), 0x7FFFFFFF, None, AND
                    ).then_inc(dve_sem, 1)
                elif kind == "addand0":
                    vector.wait_ge(xt_sem, 16)
                    vector.wait_ge(in_sem[0], 16)
                    vector.tensor_add(
                        wt[0][0:1, 0:2], wt[0][0:1, 0:2], xt[0:1, 0:2]
                    ).then_inc(add_sem, 1)
                    vector.wait_ge(add_sem, 1)  # same-engine RAW via DVE pipe
                    vector.tensor_scalar(
                        at[0].bitcast(i32), wt[0].bitcast(i32), 0x7FFFFFFF, None, AND
                    ).then_inc(dve_sem, 1)
                else:
                    vector.wait_ge(act_sem, 2 * pos[j] + 1)  # st_j ready
                    vector.tensor_mul(s2[j], st[j], st[j]).then_inc(dve_sem, 1)
                    vector.wait_ge(dve_sem, DS2[j])  # same-engine RAW (s2)
                    vector.tensor_scalar(
                        ot[j], s2[j], -2.0, 1.0, MULT, ADD
                    ).then_inc(dve_sem, 1)

    nc.finalize()
    return nc


def _get_nc():
    if "nc" not in _cache:
        _cache["nc"] = _build_nc_raw()
    return _cache["nc"]


def _run(x, w, **spmd_kwargs):
    """Shard, run on 8 cores, return (c, sneg, cosv) full f32 vectors plus
    the raw BassKernelResults (for profiling from test harnesses)."""
    from concourse.bass_utils import run_bass_kernel_spmd

    x = np.ascontiguousarray(np.asarray(x, dtype=np.float32)).reshape(1, 2)
    w = np.asarray(w, dtype=np.float32).reshape(-1)
    assert w.shape[0] == N
    w_pad = np.zeros(N_PAD, dtype=np.float32)
    w_pad[:N] = w
    shards = w_pad.reshape(N_CORES, P, F)
    zero_x = np.zeros((1, 2), dtype=np.float32)
    in_maps = [
        {"w": shards[i], "x": (x if i == 0 else zero_x)} for i in range(N_CORES)
    ]
    res = run_bass_kernel_spmd(_get_nc(), in_maps, list(range(N_CORES)), **spmd_kwargs)
    c = np.concatenate([r["c"].reshape(-1) for r in res.results])[:N]
    sneg = np.concatenate([r["s"].reshape(-1) for r in res.results])[:N]
    cosv = np.concatenate([r["o"].reshape(-1) for r in res.results])[:N]
    return c, sneg, cosv, res


def kernel(x, w):
    c, sneg, cosv, _ = _run(x, w)
    state = np.zeros((N, 4), dtype=np.float32)
    state[:, 0] = c
    state[:, 3] = sneg
    state = state.view(np.complex64).reshape(N, 2, 1)
    O = np.zeros((N, 2), dtype=np.float32)
    O[:, 0] = cosv
    O = O.view(np.complex64).reshape(N, 1, 1)
    return state, O


# revision 46
# speedup vs baseline: 1.0483x; 1.0435x over previous
"""Trainium2 Bass kernel for nn_BasicModel_47871705481510.

Math: per qubit i, with v_i = w_i + (x[0] if i==0 else x[1] if i==1 else 0):
  state_i = [cos(v_i/2), -i*sin(v_i/2)]^T   (Rx(w) @ Rx(theta1) |0> collapses
                                             to a single rotation by v_i)
  O_i     = cos(v_i)                         (real)

Device computes three f32 planes per qubit. The ACT Sin table is accurate
only for |arg| <= pi (max |v| here is ~5.42), so:
  sneg = sin(-v/2)                 |v/2| <= 2.71 < pi         (ACT)
  c    = sin(pi/2 - |v|/2)         arg in [-1.14, pi/2]       (DVE sign-clear
                                                               + ACT)
  cosv = 1 - 2*sneg^2                                          (DVE)
Host assembles the complex64 outputs (structural zeros / interleave only).

Sharding: data-parallel over qubits. 2,000,000 qubits padded to 8 * 128 * 2048
= 2,097,152; each of the 8 cores handles one contiguous [128, 2048] f32 shard.
x is passed as a [1,2] tensor: the real x to core 0 (whose shard holds qubits
0,1 at partition 0, cols 0,1), zeros to the other cores, so the SPMD program
is uniform. A [128,1] pi/2 bias column is shipped as a tiny constant input.
"""

import numpy as np

N = 2_000_000
N_CORES = 8
P = 128
F = 2048
PER_CORE = P * F  # 262144
N_PAD = PER_CORE * N_CORES

# compute chunks (columns); processed in ORDER so that chunk 0 (the only one
# needing the x-add, whose tiny DMA chain wakes slowly) runs last
CHS = [128, 512, 704, 704]
ORDER = [1, 2, 3, 0]

_cache = {}


def _build_nc_raw():
    """Raw Bacc kernel (no TileContext): hand-placed semaphores, distinct
    SBUF buffers (no reuse -> only RAW deps need sems), no Tile tail
    machinery. Streams:
      Sync:   pi/2-bias + x DMAs first (tiny, warms this queue), then the
              first two processed in-chunks, then o-plane outs
      Scalar: warm-up Sin (hoists ACT_TABLE_LOAD), remaining in-chunks,
              all ACT ops, c-plane outs
      Pool:   s-plane outs (SWDGE queue)
      Vector: per chunk: |v| sign-clear, sneg^2, cosv affine; the x add
    Outputs stream out per chunk as soon as each plane piece is computed.
    """
    import concourse.bacc as bacc
    import concourse.mybir as mybir

    nc = bacc.Bacc(
        "TRN2", target_bir_lowering=False, debug=False, num_devices=N_CORES
    )
    w_in = nc.declare_dram_parameter("w", [P, F], mybir.dt.float32, isOutput=False)
    x_in = nc.declare_dram_parameter("x", [1, 2], mybir.dt.float32, isOutput=False)
    c_out = nc.declare_dram_parameter("c", [P, F], mybir.dt.float32, isOutput=True)
    s_out = nc.declare_dram_parameter("s", [P, F], mybir.dt.float32, isOutput=True)
    o_out = nc.declare_dram_parameter("o", [P, F], mybir.dt.float32, isOutput=True)

    SIN = mybir.ActivationFunctionType.Sin
    MULT = mybir.AluOpType.mult
    ADD = mybir.AluOpType.add
    AND = mybir.AluOpType.bitwise_and
    f32 = mybir.dt.float32
    i32 = mybir.dt.int32

    NCH = len(CHS)
    offs = [sum(CHS[:j]) for j in range(NCH)]
    assert sum(CHS) == F
    assert sorted(ORDER) == list(range(NCH)) and ORDER[-1] == 0
    # processing position of chunk j
    pos = {j: p for p, j in enumerate(ORDER)}

    wt = [nc.alloc_sbuf_tensor(f"wt{j}", [P, CHS[j]], f32).ap() for j in range(NCH)]
    at = [nc.alloc_sbuf_tensor(f"at{j}", [P, CHS[j]], f32).ap() for j in range(NCH)]
    s2 = [nc.alloc_sbuf_tensor(f"s2{j}", [P, CHS[j]], f32).ap() for j in range(NCH)]
    st = [nc.alloc_sbuf_tensor(f"st{j}", [P, CHS[j]], f32).ap() for j in range(NCH)]
    ct = [nc.alloc_sbuf_tensor(f"ct{j}", [P, CHS[j]], f32).ap() for j in range(NCH)]
    ot = [nc.alloc_sbuf_tensor(f"ot{j}", [P, CHS[j]], f32).ap() for j in range(NCH)]
    xt = nc.alloc_sbuf_tensor("xt", [1, 2], f32).ap()
    bt = nc.alloc_sbuf_tensor("bt", [P, 1], f32).ap()
    warm = nc.alloc_sbuf_tensor("warm", [1, 1], f32).ap()
    zconst = nc.const_aps.tensor(0.0, (1, 1), f32)

    in_sl = [slice(offs[j], offs[j] + CHS[j]) for j in range(NCH)]

    # semaphore counts by processing position p (chunk j = ORDER[p]):
    #   act: st -> 2p+1, c -> 2p+2
    # DVE stream: ANDs for chunks ORDER[:-1] hoisted to the front (they only
    # need input data); the x add + AND_0 slot in right after the first
    # [s2, o] pair so chunk 0's ACT work can append immediately after the
    # other chunks'; remaining [s2, o] pairs follow.
    DVE_SEQ = (
        [("and", j) for j in ORDER[:-1]]
        + [("s2o", ORDER[0]), ("addand0", 0)]
        + [("s2o", j) for j in ORDER[1:-1]]
        + [("s2o", 0)]
    )
    DAND = {}
    DS2 = {}
    DOT = {}
    _d = 0
    for kind, j in DVE_SEQ:
        if kind == "and":
            _d += 1
            DAND[j] = _d
        elif kind == "addand0":
            _d += 1
            DAND[0] = _d
        else:
            _d += 1
            DS2[j] = _d
            _d += 1
            DOT[j] = _d

    from contextlib import ExitStack

    with ExitStack() as stack:
        in_sem = [stack.enter_context(nc.semaphore(f"in{j}")) for j in range(NCH)]
        xt_sem = stack.enter_context(nc.semaphore("xt_sem"))
        bt_sem = stack.enter_context(nc.semaphore("bt_sem"))
        act_sem = stack.enter_context(nc.semaphore("act_sem"))
        add_sem = stack.enter_context(nc.semaphore("add_sem"))
        dve_sem = stack.enter_context(nc.semaphore("dve_sem"))
        out_sc = stack.enter_context(nc.semaphore("out_sc"))
        out_sy = stack.enter_context(nc.semaphore("out_sy"))
        out_po = stack.enter_context(nc.semaphore("out_po"))
        block = stack.enter_context(nc.Block())

        @block.sync
        def _(sync):
            # first processed in-chunk FIRST: it gates the whole pipeline;
            # the tiny x transfer rides behind it
            sync.dma_start(
                wt[ORDER[0]], w_in[:, in_sl[ORDER[0]]]
            ).then_inc(in_sem[ORDER[0]], 16)
            sync.dma_start(xt, x_in[:]).then_inc(xt_sem, 16)
            for p, j in enumerate(ORDER):
                sync.wait_ge(dve_sem, DOT[j])
                sync.dma_start(o_out[:, in_sl[j]], ot[j]).then_inc(out_sy, 16)
            sync.wait_ge(out_sy, 16 * NCH)

        @block.scalar
        def _(scalar):
            scalar.activation(warm, zconst, SIN)  # pulls ACT_TABLE_LOAD early
            for j in ORDER[1:]:
                scalar.dma_start(wt[j], w_in[:, in_sl[j]]).then_inc(in_sem[j], 16)
            for p, j in enumerate(ORDER):
                scalar.wait_ge(in_sem[j], 16)
                if j == 0:
                    scalar.wait_ge(add_sem, 1)
                scalar.activation(st[j], wt[j], SIN, scale=-0.5).then_inc(act_sem, 1)
                if p == 0:
                    scalar.wait_ge(bt_sem, 1)
                scalar.wait_ge(dve_sem, DAND[j])  # |v| ready
                scalar.activation(ct[j], at[j], SIN, bias=bt, scale=-0.5).then_inc(
                    act_sem, 1
                )
                if p >= 1:
                    # previous chunk's c piece out: the DIRECT2D (~0.65us)
                    # hides under this chunk's ACT execution
                    jprev = ORDER[p - 1]
                    scalar.wait_ge(act_sem, 2 * p)
                    scalar.dma_start(
                        c_out[:, in_sl[jprev]], ct[jprev]
                    ).then_inc(out_sc, 16)
            scalar.wait_ge(act_sem, 2 * NCH)
            scalar.dma_start(
                c_out[:, in_sl[ORDER[-1]]], ct[ORDER[-1]]
            ).then_inc(out_sc, 16)
            scalar.wait_ge(out_sc, 16 * NCH)

        @block.gpsimd
        def _(gpsimd):
            # pi/2 bias column produced locally -- no DMA, no latency chain
            gpsimd.memset(bt, float(np.pi / 2)).then_inc(bt_sem, 1)
            for p, j in enumerate(ORDER):
                gpsimd.wait_ge(act_sem, 2 * p + 1)
                gpsimd.dma_start(s_out[:, in_sl[j]], st[j]).then_inc(out_po, 16)
            gpsimd.wait_ge(out_po, 16 * NCH)

        @block.vector
        def _(vector):
            for kind, j in DVE_SEQ:
                if kind == "and":
                    vector.wait_ge(in_sem[j], 16)
                    vector.tensor_scalar(
                        at[j].bitcast(i32), wt[j].bitcast(i32), 0x7FFFFFFF, None, AND
                    ).then_inc(dve_sem, 1)
                elif kind == "addand0":
                    vector.wait_ge(xt_sem, 16)
                    vector.wait_ge(in_sem[0], 16)
                    vector.tensor_add(
                        wt[0][0:1, 0:2], wt[0][0:1, 0:2], xt[0:1, 0:2]
                    ).then_inc(add_sem, 1)
                    vector.wait_ge(add_sem, 1)  # same-engine RAW via DVE pipe
                    vector.tensor_scalar(
                        at[0].bitcast(i32), wt[0].bitcast(i32), 0x7FFFFFFF, None, AND
                    ).then_inc(dve_sem, 1)
                else:
                    vector.wait_ge(act_sem, 2 * pos[j] + 1)  # st_j ready
                    vector.tensor_mul(s2[j], st[j], st[j]).then_inc(dve_sem, 1)
                    vector.wait_ge(dve_sem, DS2[j])  # same-engine RAW (s2)
                    vector.tensor_scalar(
                        ot[j], s2[j], -2.0, 1.0, MULT, ADD
                    ).then_inc(dve_sem, 1)

    nc.finalize()
    return nc


def _get_nc():
    if "nc" not in _cache:
        _cache["nc"] = _build_nc_raw()
    return _cache["nc"]


def _run(x, w, **spmd_kwargs):
    """Shard, run on 8 cores, return (c, sneg, cosv) full f32 vectors plus
    the raw BassKernelResults (for profiling from test harnesses)."""
    from concourse.bass_utils import run_bass_kernel_spmd

    x = np.ascontiguousarray(np.asarray(x, dtype=np.float32)).reshape(1, 2)
    w = np.asarray(w, dtype=np.float32).reshape(-1)
    assert w.shape[0] == N
    w_pad = np.zeros(N_PAD, dtype=np.float32)
    w_pad[:N] = w
    shards = w_pad.reshape(N_CORES, P, F)
    zero_x = np.zeros((1, 2), dtype=np.float32)
    in_maps = [
        {"w": shards[i], "x": (x if i == 0 else zero_x)} for i in range(N_CORES)
    ]
    res = run_bass_kernel_spmd(_get_nc(), in_maps, list(range(N_CORES)), **spmd_kwargs)
    c = np.concatenate([r["c"].reshape(-1) for r in res.results])[:N]
    sneg = np.concatenate([r["s"].reshape(-1) for r in res.results])[:N]
    cosv = np.concatenate([r["o"].reshape(-1) for r in res.results])[:N]
    return c, sneg, cosv, res


def kernel(x, w):
    c, sneg, cosv, _ = _run(x, w)
    state = np.zeros((N, 4), dtype=np.float32)
    state[:, 0] = c
    state[:, 3] = sneg
    state = state.view(np.complex64).reshape(N, 2, 1)
    O = np.zeros((N, 2), dtype=np.float32)
    O[:, 0] = cosv
    O = O.view(np.complex64).reshape(N, 1, 1)
    return state, O
